# revision 7
# baseline (speedup 1.0000x reference)
"""ConvCNP encoder kernel for 8x TRN2 NeuronCores — point-sharded version.

Math: for a 128x128 uniform grid g=(xs[i], ys[j]) and n=8192 data points X
(2-D) with values psi(Y) = [1, Y0, Y1]:

    Gram[g, x] = exp(-0.5*||g - X[x]||^2)
    fm = Gram @ psi                  # (G, 3); column 0 == row-sum (denominator)
    out[c, j, i] = fm[(i, j), c], with c=1,2 normalized by column 0.

The squared distance is separable over the grid axes:

    Gram[(i,j), x] = A[i, x] * B[j, x]
      A[i, x] = exp(-0.5*(xs[i] - X0[x])^2)     B[j, x] = exp(-0.5*(ys[j] - X1[x])^2)

Sharding: the 8192 POINTS are split across the 8 cores (1024 each); the grid
is replicated. Each core computes a partial fm over its point slice; the host
sums the 8 partials (the unshard step), then normalizes channels 1,2.
This makes every stage 8x smaller than grid sharding, which had to
recompute the full A factor on every core.

Per core (8 contraction chunks of 128 points):

    acc[i, (c,j)] = sum_k  AT_k^T @ BfT_k        (PE, fp16, PSUM accum)
      AT_k  = exp(-0.5*(xs[i]  - X0)^2)   [x_part=128, i=128]
      BfT_k = [B | B*Y0 | B*Y1]           [x_part=128, 3*128]

The exp ARGUMENT is produced on the PE (not DVE): with block-diagonal
constants, one K=9 matmul per 4-chunk group computes
    x*xs[i] - 0.5*xs[i]^2 - 0.5*x^2  =  -0.5*(xs[i] - x)^2
for 4 chunks at once (N=512). The four group matmuls go to distinct
32-aligned row groups via tile_position, so they run concurrently.
The -0.5*x^2 rows are the x rows squared in place by the DVE (host only
does layout). Folding the full quadratic into the argument keeps both
factors <= 1, so fp16 never overflows and psi needs only 2 multiplies.
"""

import numpy as np
from contextlib import ExitStack

N_AXIS = 128          # grid points per axis
NPTS = 8192           # data points
NCORES = 8
PTS = NPTS // NCORES   # 1024 points per core
NCHUNK = PTS // 128    # 8 contraction chunks of 128
NGROUP = 2             # chunk groups of 4 (one sq-matmul each per axis)
GRID_LO, GRID_HI = -2.0, 2.0

_CACHE = {}


def _patch_walrus_flags():
    """Cap the compiler's semaphore file so the NEFF epilogue restores fewer
    semaphores (the restore is ~40ns/sem/engine of pure tail latency).
    Idempotent."""
    import concourse.bass_utils as bu

    if getattr(bu.run_command, "_sem_cap_patched", False):
        return
    orig = bu.run_command

    def run_command_capped(argv, **kwargs):
        if argv and "walrus_driver" in str(argv[0]) and any(
                str(a).startswith("--neff-output-filename") for a in argv):
            argv = list(argv) + ["--max-sem-num=176"]
        return orig(argv, **kwargs)

    run_command_capped._sem_cap_patched = True
    bu.run_command = run_command_capped


def _build_program():
    import concourse.bacc as bacc
    import concourse.mybir as mybir
    import concourse.tile as tile

    _patch_walrus_flags()

    f32 = mybir.dt.float32
    f16 = mybir.dt.float16
    nc = bacc.Bacc("TRN2", target_bir_lowering=False, debug=False, num_devices=NCORES,
                   enable_partition_id=False, monotonic_sem_count=0)

    # Inputs:
    #   sa [128, 128] f16: per-group lhsT rows at base partitions 0/32/64/96:
    #        b+0..3: x rows (squared in place on device -> -0.5*x^2 term),
    #        b+4..7: x rows, b+8: ones
    #   cr [128, 512] f16: constant rhs, same 9-row pattern at each base b:
    #        b+kk: -0.5 at block kk; b+4+kk: xs at block kk; b+8: -0.5*xs^2 tiled
    #   yc [128, 16] f16: Y0t (cols 0:8) | Y1t (cols 8:16), [x_part, chunk]
    sa = nc.dram_tensor("sa", [128, 128], f16, kind="ExternalInput")
    cr = nc.dram_tensor("cr", [128, 512], f16, kind="ExternalInput")
    yc = nc.dram_tensor("yc", [128, 16], f16, kind="ExternalInput")
    out = nc.dram_tensor("out", [128, 3 * N_AXIS], f16, kind="ExternalOutput")

    BASES = (0, 32, 64, 96)  # A-g0, A-g1, B-g0, B-g1

    with tile.TileContext(nc) as tc, ExitStack() as ctx:
        singles = ctx.enter_context(tc.tile_pool(name="singles", bufs=1))
        psum = ctx.enter_context(tc.tile_pool(name="psum", bufs=1, space="PSUM"))

        s_sa = singles.tile([128, 128], f16, tag="sa")
        nc.sync.dma_start(s_sa[:, :], sa[:, :])
        s_cr = singles.tile([128, 512], f16, tag="cr")
        nc.sync.dma_start(s_cr[:, :], cr[:, :])
        s_yc = singles.tile([128, 16], f16, tag="yc")
        nc.gpsimd.dma_start(s_yc[:, :], yc[:, :])

        # Square the duplicated x rows in place (rows b..b+3; engine APs
        # need 32-aligned base partitions, so the squared rows lead).
        for b in BASES:
            nc.vector.tensor_tensor(
                s_sa[b:b + 4, :], s_sa[b:b + 4, :], s_sa[b:b + 4, :],
                mybir.AluOpType.mult,
            )

        # One K=9 matmul per group: psq[g] [128, 4, 128] fp32 holds
        # -0.5*(axis - x)^2 for 4 chunks. Distinct row groups -> concurrent.
        psq = [psum.tile([128, 4, 128], f32, tag=f"psq{gi}", name=f"psq{gi}")
               for gi in range(4)]
        for gi, b in enumerate(BASES):
            nc.tensor.matmul(
                psq[gi][:, :, :],
                s_sa[b:b + 9, :],
                s_cr[b:b + 9, :],
                start=True, stop=True,
                tile_position=(b, 0),
            )

        # exp: A groups -> at [128, 8, 128]; B groups -> column block 0 of
        # bf [128, 8, 384] (so chunk k's full 384-wide rhs is contiguous).
        at = singles.tile([128, NCHUNK, 128], f16, tag="at")
        bf = singles.tile([128, NCHUNK, 3 * 128], f16, tag="bf")
        acc = psum.tile([128, 3 * N_AXIS], f32, tag="acc")

        for g in range(NGROUP):
            k0 = 4 * g
            nc.scalar.activation(
                bf[:, k0:k0 + 4, 0:128], psq[2 + g][:, :, :],
                mybir.ActivationFunctionType.Exp,
            )
            nc.scalar.activation(
                at[:, k0:k0 + 4, :], psq[g][:, :, :],
                mybir.ActivationFunctionType.Exp,
            )
            for c in (1, 2):
                nc.vector.tensor_tensor(
                    bf[:, k0:k0 + 4, c * 128:(c + 1) * 128],
                    bf[:, k0:k0 + 4, 0:128],
                    s_yc[:, (c - 1) * 8 + k0:(c - 1) * 8 + k0 + 4]
                        .unsqueeze(2).broadcast_to([128, 4, 128]),
                    mybir.AluOpType.mult,
                )
            for k in range(k0, k0 + 4):
                nc.tensor.matmul(
                    acc[:, :],
                    at[:, k:k + 1, :],       # stationary lhsT [128, 128] f16
                    bf[:, k:k + 1, :],       # moving rhs [128, 384] f16
                    start=(k == 0),
                    stop=(k == NCHUNK - 1),
                )

        s_out = singles.tile([128, 3 * N_AXIS], f16, tag="outt")
        nc.scalar.activation(
            s_out[:, :], acc[:, :], mybir.ActivationFunctionType.Copy,
        )
        nc.sync.dma_start(out[:, :], s_out[:, :])

    nc.finalize()
    return nc


def _get_program():
    if "nc" not in _CACHE:
        _CACHE["nc"] = _build_program()
    return _CACHE["nc"]


def _host_inputs(X, Y):
    """Build the per-core input maps (layout prep only)."""
    X = np.ascontiguousarray(np.asarray(X, dtype=np.float32))
    Y = np.ascontiguousarray(np.asarray(Y, dtype=np.float32))
    xs = np.linspace(GRID_LO, GRID_HI, N_AXIS, dtype=np.float32)

    cr = np.zeros((128, 512), np.float32)
    for b in (0, 32, 64, 96):
        for kk in range(4):
            cr[b + kk, kk * 128:(kk + 1) * 128] = -0.5
            cr[b + 4 + kk, kk * 128:(kk + 1) * 128] = xs
        cr[b + 8, :] = np.tile(-0.5 * xs * xs, 4)
    cr = cr.astype(np.float16)

    in_maps = []
    for m in range(NCORES):
        sl = slice(m * PTS, (m + 1) * PTS)
        x0 = X[sl, 0].reshape(NCHUNK, 128)
        x1 = X[sl, 1].reshape(NCHUNK, 128)
        sa = np.zeros((128, 128), np.float32)
        for b, rows in zip((0, 32, 64, 96),
                           (x0[0:4], x0[4:8], x1[0:4], x1[4:8])):
            sa[b:b + 4] = rows
            sa[b + 4:b + 8] = rows
            sa[b + 8] = 1.0
        yc = np.empty((128, 16), np.float16)
        yc[:, 0:8] = Y[sl, 0].reshape(NCHUNK, 128).T
        yc[:, 8:16] = Y[sl, 1].reshape(NCHUNK, 128).T
        in_maps.append({"sa": sa.astype(np.float16), "cr": cr, "yc": yc})
    return in_maps


def run_on_cores(X, Y, **spmd_kwargs):
    """Run the SPMD kernel; returns BassKernelResults."""
    from concourse.bass_utils import run_bass_kernel_spmd

    nc = _get_program()
    in_maps = _host_inputs(X, Y)
    res = run_bass_kernel_spmd(nc, in_maps, core_ids=list(range(NCORES)),
                               **spmd_kwargs)
    return res


def kernel(X, Y):
    res = run_on_cores(X, Y)
    # Unshard: sum the per-core partial feature maps, then normalize.
    acc = np.zeros((3, N_AXIS, N_AXIS), dtype=np.float32)
    for r in res.results:
        blk = r["out"].astype(np.float32)       # [i, (c, j)]
        acc += blk.reshape(N_AXIS, 3, N_AXIS).transpose(1, 2, 0)  # -> [c, j, i]
    full = np.empty_like(acc)
    full[0] = acc[0]
    full[1] = acc[1] / acc[0]
    full[2] = acc[2] / acc[0]
    return full


# revision 8
# speedup vs baseline: 1.0081x; 1.0081x over previous
"""ConvCNP encoder kernel for 8x TRN2 NeuronCores — point-sharded version.

Math: for a 128x128 uniform grid g=(xs[i], ys[j]) and n=8192 data points X
(2-D) with values psi(Y) = [1, Y0, Y1]:

    Gram[g, x] = exp(-0.5*||g - X[x]||^2)
    fm = Gram @ psi                  # (G, 3); column 0 == row-sum (denominator)
    out[c, j, i] = fm[(i, j), c], with c=1,2 normalized by column 0.

The squared distance is separable over the grid axes:

    Gram[(i,j), x] = A[i, x] * B[j, x]
      A[i, x] = exp(-0.5*(xs[i] - X0[x])^2)     B[j, x] = exp(-0.5*(ys[j] - X1[x])^2)

Sharding: the 8192 POINTS are split across the 8 cores (1024 each); the grid
is replicated. Each core computes a partial fm over its point slice; the host
sums the 8 partials (the unshard step), then normalizes channels 1,2.
This makes every stage 8x smaller than grid sharding, which had to
recompute the full A factor on every core.

Per core (8 contraction chunks of 128 points):

    acc[i, (c,j)] = sum_k  AT_k^T @ BfT_k        (PE, fp8 DoubleRow, PSUM accum)
      AT_k  = exp(-0.5*(xs[i]  - X0)^2)   [x_part=128, i=128]
      BfT_k = [B | B*Y0 | B*Y1]           [x_part=128, 3*128]

The exp ARGUMENT is produced on the PE (not DVE): with block-diagonal
constants, one K=9 matmul per 4-chunk group computes
    x*xs[i] - 0.5*xs[i]^2 - 0.5*x^2  =  -0.5*(xs[i] - x)^2
for 4 chunks at once (N=512). The four group matmuls go to distinct
32-aligned row groups via tile_position, so they run concurrently.
The -0.5*x^2 lhsT rows are the x rows squared in place (DVE for the A
groups, ACT Square for the B groups — two engines in parallel; host only
does layout). Folding the full quadratic into the argument keeps both
factors <= 1 (no overflow in 8-bit) and psi needs only 2 multiplies.

The accumulation matmuls run in fp8e4m3 with perf_mode=DoubleRow: chunk
PAIRS are interleaved as [Ki=128, Ko=2, N], virtualizing a 128x256 array
— 4 matmuls instead of 8, ~2x the ALU rate. PSUM accumulates fp32.
All inputs arrive in ONE packed DMA (one issue + one completion wait).
"""

import numpy as np
from contextlib import ExitStack

N_AXIS = 128          # grid points per axis
NPTS = 8192           # data points
NCORES = 8
PTS = NPTS // NCORES   # 1024 points per core
NCHUNK = PTS // 128    # 8 contraction chunks of 128
GRID_LO, GRID_HI = -2.0, 2.0
USE_FP8 = True         # fp8e4m3 + DoubleRow for the accumulation matmuls

_CACHE = {}


def _patch_walrus_flags():
    """Cap the compiler's semaphore allocation (epilogue restore tail).
    Idempotent."""
    import concourse.bass_utils as bu

    if getattr(bu.run_command, "_sem_cap_patched", False):
        return
    orig = bu.run_command

    def run_command_capped(argv, **kwargs):
        if argv and "walrus_driver" in str(argv[0]) and any(
                str(a).startswith("--neff-output-filename") for a in argv):
            argv = list(argv) + ["--max-sem-num=176"]
        return orig(argv, **kwargs)

    run_command_capped._sem_cap_patched = True
    bu.run_command = run_command_capped


def _build_program():
    import concourse.bacc as bacc
    import concourse.mybir as mybir
    import concourse.tile as tile

    _patch_walrus_flags()

    f32 = mybir.dt.float32
    f16 = mybir.dt.float16
    f8 = mybir.dt.float8e4
    gdt = f8 if USE_FP8 else f16
    nc = bacc.Bacc("TRN2", target_bir_lowering=False, debug=False, num_devices=NCORES,
                   enable_partition_id=False, monotonic_sem_count=0)

    # One packed input [128, 656] f16:
    #   cols 0:128   sa: per-group lhsT rows at base partitions 0/32/64/96:
    #                b+0..3: x rows (squared in place -> -0.5*x^2 term),
    #                b+4..7: x rows, b+8: ones
    #   cols 128:640 cr: constant rhs, same 9-row pattern at each base b:
    #                b+kk: -0.5 at block kk; b+4+kk: xs at block kk;
    #                b+8: -0.5*xs^2 tiled
    #   cols 640:656 yc: Y0t (8) | Y1t (8), [x_part, chunk]
    inp = nc.dram_tensor("inp", [128, 656], f16, kind="ExternalInput")
    out = nc.dram_tensor("out", [128, 3 * N_AXIS], f16, kind="ExternalOutput")

    BASES = (0, 32, 64, 96)  # A-g0, A-g1, B-g0, B-g1

    with tile.TileContext(nc) as tc, ExitStack() as ctx:
        singles = ctx.enter_context(tc.tile_pool(name="singles", bufs=1))
        psum = ctx.enter_context(tc.tile_pool(name="psum", bufs=1, space="PSUM"))

        s_in = singles.tile([128, 656], f16, tag="inp")
        nc.sync.dma_start(s_in[:, :], inp[:, :])

        # Square the duplicated x rows in place (rows b..b+3). A groups on
        # DVE, B groups on ACT Square — two engines in parallel.
        for b in (0, 32):
            nc.vector.tensor_tensor(
                s_in[b:b + 4, 0:128], s_in[b:b + 4, 0:128], s_in[b:b + 4, 0:128],
                mybir.AluOpType.mult,
            )
        for b in (64, 96):
            nc.scalar.activation(
                s_in[b:b + 4, 0:128], s_in[b:b + 4, 0:128],
                mybir.ActivationFunctionType.Square,
            )

        # One K=9 matmul per group: psq[g] [128, 4, 128] fp32 holds
        # -0.5*(axis - x)^2 for 4 chunks. Distinct row groups -> concurrent.
        psq = [psum.tile([128, 4, 128], f32, tag=f"psq{gi}", name=f"psq{gi}")
               for gi in range(4)]
        for gi, b in enumerate(BASES):
            nc.tensor.matmul(
                psq[gi][:, :, :],
                s_in[b:b + 9, 0:128],
                s_in[b:b + 9, 128:640],
                start=True, stop=True,
                tile_position=(b, 0),
            )

        # exp: A groups -> at [128, 8, 128]; B groups -> column block 0 of
        # bf [128, 8, 384] (chunk k's full 384-wide rhs stays contiguous).
        at = singles.tile([128, NCHUNK, 128], gdt, tag="at")
        bf = singles.tile([128, NCHUNK, 3 * 128], gdt, tag="bf")
        acc = psum.tile([128, 3 * N_AXIS], f32, tag="acc")

        for g in range(2):
            k0 = 4 * g
            nc.scalar.activation(
                bf[:, k0:k0 + 4, 0:128], psq[2 + g][:, :, :],
                mybir.ActivationFunctionType.Exp,
            )
            nc.scalar.activation(
                at[:, k0:k0 + 4, :], psq[g][:, :, :],
                mybir.ActivationFunctionType.Exp,
            )
            for c in (1, 2):
                nc.vector.tensor_tensor(
                    bf[:, k0:k0 + 4, c * 128:(c + 1) * 128],
                    bf[:, k0:k0 + 4, 0:128],
                    s_in[:, 640 + (c - 1) * 8 + k0:640 + (c - 1) * 8 + k0 + 4]
                        .unsqueeze(2).broadcast_to([128, 4, 128]),
                    mybir.AluOpType.mult,
                )
            if USE_FP8:
                for p in (0, 1):
                    k = k0 + 2 * p
                    nc.tensor.matmul(
                        acc[:, :],
                        at[:, k:k + 2, :],       # [Ki=128, Ko=2, 128] fp8
                        bf[:, k:k + 2, :],       # [Ki=128, Ko=2, 384] fp8
                        start=(k == 0),
                        stop=(k == NCHUNK - 2),
                        perf_mode=mybir.MatmulPerfMode.DoubleRow,
                    )
            else:
                for k in range(k0, k0 + 4):
                    nc.tensor.matmul(
                        acc[:, :],
                        at[:, k:k + 1, :],
                        bf[:, k:k + 1, :],
                        start=(k == 0),
                        stop=(k == NCHUNK - 1),
                    )

        s_out = singles.tile([128, 3 * N_AXIS], f16, tag="outt")
        nc.scalar.activation(
            s_out[:, :], acc[:, :], mybir.ActivationFunctionType.Copy,
        )
        nc.sync.dma_start(out[:, :], s_out[:, :])

    nc.finalize()
    return nc


def _get_program():
    if "nc" not in _CACHE:
        _CACHE["nc"] = _build_program()
    return _CACHE["nc"]


def _host_inputs(X, Y):
    """Build the per-core input maps (layout prep only)."""
    X = np.ascontiguousarray(np.asarray(X, dtype=np.float32))
    Y = np.ascontiguousarray(np.asarray(Y, dtype=np.float32))
    xs = np.linspace(GRID_LO, GRID_HI, N_AXIS, dtype=np.float32)

    cr = np.zeros((128, 512), np.float32)
    for b in (0, 32, 64, 96):
        for kk in range(4):
            cr[b + kk, kk * 128:(kk + 1) * 128] = -0.5
            cr[b + 4 + kk, kk * 128:(kk + 1) * 128] = xs
        cr[b + 8, :] = np.tile(-0.5 * xs * xs, 4)

    in_maps = []
    for m in range(NCORES):
        sl = slice(m * PTS, (m + 1) * PTS)
        x0 = X[sl, 0].reshape(NCHUNK, 128)
        x1 = X[sl, 1].reshape(NCHUNK, 128)
        inp = np.zeros((128, 656), np.float32)
        inp[:, 128:640] = cr
        for b, rows in zip((0, 32, 64, 96),
                           (x0[0:4], x0[4:8], x1[0:4], x1[4:8])):
            inp[b:b + 4, 0:128] = rows
            inp[b + 4:b + 8, 0:128] = rows
            inp[b + 8, 0:128] = 1.0
        inp[:, 640:648] = Y[sl, 0].reshape(NCHUNK, 128).T
        inp[:, 648:656] = Y[sl, 1].reshape(NCHUNK, 128).T
        in_maps.append({"inp": inp.astype(np.float16)})
    return in_maps


def run_on_cores(X, Y, **spmd_kwargs):
    """Run the SPMD kernel; returns BassKernelResults."""
    from concourse.bass_utils import run_bass_kernel_spmd

    nc = _get_program()
    in_maps = _host_inputs(X, Y)
    res = run_bass_kernel_spmd(nc, in_maps, core_ids=list(range(NCORES)),
                               **spmd_kwargs)
    return res


def kernel(X, Y):
    res = run_on_cores(X, Y)
    # Unshard: sum the per-core partial feature maps, then normalize.
    acc = np.zeros((3, N_AXIS, N_AXIS), dtype=np.float32)
    for r in res.results:
        blk = r["out"].astype(np.float32)       # [i, (c, j)]
        acc += blk.reshape(N_AXIS, 3, N_AXIS).transpose(1, 2, 0)  # -> [c, j, i]
    full = np.empty_like(acc)
    full[0] = acc[0]
    full[1] = acc[1] / acc[0]
    full[2] = acc[2] / acc[0]
    return full
